# revision 10
# baseline (speedup 1.0000x reference)
"""Trainium2 Bass kernel for nn_DiffAttention (GNN message passing), v2.

Math (per edge i: src s_i -> dst n, dst sorted):
  d_i = (h_dst[n] - h_src[s_i]) @ W_fc.T ;  e_i = tanh(d_i @ w_attn)
  alpha = segment_softmax(e, dst);  out[n] = elu(sum_i alpha_i d_i)
Since e in [-1,1], softmax needs no max-subtraction:
  out[n] = elu(p_dst[n] - (sum_i w_i p_src[s_i]) / (sum_i w_i)),
  w_i = exp(tanh(q_dst[n] - q_src[s_i])), p = h @ W_fc.T, q = p @ w_attn.

v2 device strategy (8 cores, SPMD, dst-node-range sharding):
  - fp16 node tables, 256B rows (the dma_gather granularity):
    src table [NPAD, 128] = [p(64) | 1 | q | pad62], AllGathered (Shared);
    dst table [SHARD, 128] core-local.
  - per-edge src rows fetched with gpsimd.dma_gather (custom SWDGE ucode,
    ~0.34ns/descriptor) instead of generic indirect DMA.  int16 gather
    indices only span 32K rows, so each 2048-slot window is split into 4
    node-quadrant buckets with fixed 512-slot regions.
  - per window (<=128 dst nodes):  4 gathers; one fused DVE op per
    128-edge tile produces masked q_dst broadcast + row-sum (qd) via
    accum_out; batched tanh/exp; one DVE op builds the w-scaled one-hot
    which feeds PSUM matmul accumulation of [sum w*p | sum w].
  - window dst rows for all windows are prefetch-gathered in the prologue.
Host does only index prep (windows, quadrant bucketing, int16 wrapping).
"""
import sys
sys.path.insert(0, "/opt/trn_rl_repo")
import numpy as np

N = 100000
D = 64
NC = 8
SHARD = 12544          # dst nodes owned per core; NC*SHARD = 100352
NPAD = NC * SHARD
QS = NPAD // 4         # 25088-row quadrants so gather indices fit int16
K = 16                 # 128-slot tiles per window
WE = K * 128           # 2048 edge slots per window
QCAP = 512             # slots per quadrant region (4 x 512 = 2048)
WIN_NODES = 128
MAIN_REPEAT = 1        # test.py overrides for timing
NQUEUE = 4


# ---------------------------------------------------------------- host prep
def _wrap16(vals, reps):
    # gather position i reads idxs[i % 16, i // 16]; replicate across the
    # 8 gpsimd partition groups.
    n = vals.shape[0]
    return np.tile(vals.reshape(n // 16, 16).T, (reps, 1))


def _prep_core(src, dst, c):
    n_lo = c * SHARD
    n_hi = min((c + 1) * SHARD, N)
    e_lo = np.searchsorted(dst, n_lo, side="left")
    e_hi = np.searchsorted(dst, n_hi, side="left")
    s = src[e_lo:e_hi]
    d = (dst[e_lo:e_hi] - n_lo).astype(np.int64)
    q = s // QS
    n_nodes = n_hi - n_lo
    cq = np.bincount(d * 4 + q, minlength=n_nodes * 4).reshape(n_nodes, 4)
    deg = cq.sum(1)
    estart = np.concatenate([[0], np.cumsum(deg)])
    assert cq.max() <= QCAP, f"node quadrant degree {cq.max()} > {QCAP}"

    windows = []
    n0 = 0
    while n0 < n_nodes:
        used = np.zeros(4, np.int64)
        n = n0
        while n < n_nodes and n - n0 < WIN_NODES:
            u2 = used + cq[n]
            if (u2 > QCAP).any():
                break
            used = u2
            n += 1
        windows.append((n0, n))
        n0 = n

    nW = len(windows)
    sidx = np.zeros((nW, 128, 128), np.int16)
    dloc = np.full((nW, 128, K), -1.0, np.float32)
    nidv = np.zeros((nW, 128), np.int16)
    bases, nns = [], []
    for w, (a, b) in enumerate(windows):
        elo, ehi = estart[a], estart[b]
        se, de, qe = s[elo:ehi], d[elo:ehi] - a, q[elo:ehi]
        order = np.argsort(qe, kind="stable")
        se, de, qe = se[order], de[order], qe[order]
        qcnt = np.bincount(qe, minlength=4)
        qst = np.concatenate([[0], np.cumsum(qcnt)])
        slot = np.arange(ehi - elo) - qst[qe] + qe * QCAP
        sv = np.zeros(WE, np.int16)
        dv = np.full(WE, -1.0, np.float32)
        sv[slot] = (se - qe * QS).astype(np.int16)
        dv[slot] = de.astype(np.float32)
        for bq in range(4):
            sidx[w, :, bq * 32:(bq + 1) * 32] = _wrap16(
                sv[bq * QCAP:(bq + 1) * QCAP], 8)
        dloc[w] = dv.reshape(K, 128).T
        nv = np.zeros(128, np.int16)
        nv[: b - a] = np.arange(a, b, dtype=np.int16)
        nidv[w] = nv
        bases.append(n_lo + a)
        nns.append(b - a)
    return dict(sidx=sidx, dloc=dloc, nidv=nidv,
                base=np.array(bases), nn=np.array(nns))


def _prep(src, dst):
    src = np.asarray(src, np.int64)
    dst = np.asarray(dst, np.int64)
    if np.any(np.diff(dst) < 0):
        order = np.argsort(dst, kind="stable")
        src, dst = src[order], dst[order]
    cores = [_prep_core(src, dst, c) for c in range(NC)]
    nW = max(c["sidx"].shape[0] for c in cores)
    for core in cores:
        w0 = core["sidx"].shape[0]
        pad = nW - w0
        if pad:
            core["sidx"] = np.concatenate(
                [core["sidx"], np.zeros((pad, 128, 128), np.int16)])
            core["dloc"] = np.concatenate(
                [core["dloc"], np.full((pad, 128, K), -1.0, np.float32)])
            core["nidv"] = np.concatenate(
                [core["nidv"], np.zeros((pad, 128), np.int16)])
            core["base"] = np.concatenate([core["base"], np.full(pad, N)])
            core["nn"] = np.concatenate([core["nn"], np.zeros(pad, np.int64)])
        # nid wrapped for the prologue batch gathers: global position
        # i = w*128 + j -> idx[i%16, i//16] -> col w*8 + j//16
        nid_all = np.zeros((128, nW * 8), np.int16)
        for w in range(nW):
            nid_all[:, w * 8:(w + 1) * 8] = _wrap16(core["nidv"][w], 8)
        core["nid_all"] = nid_all
    return cores, nW


# ---------------------------------------------------------------- device
def _build_program(nW, main_repeat, ablate=""):
    from concourse import bass, bacc, mybir, tile, library_config
    f32, f16 = mybir.dt.float32, mybir.dt.float16
    i16 = mybir.dt.int16
    EQ, MUL = mybir.AluOpType.is_equal, mybir.AluOpType.mult
    ADD, SUB = mybir.AluOpType.add, mybir.AluOpType.subtract

    nc = bacc.Bacc("TRN2", target_bir_lowering=False, debug=False,
                   num_devices=NC, num_swdge_queues=NQUEUE)
    hs_e = nc.dram_tensor("hs", [SHARD, D], f32, kind="ExternalInput")
    hd_e = nc.dram_tensor("hd", [SHARD, D], f32, kind="ExternalInput")
    wfc_e = nc.dram_tensor("wfc", [D, D], f32, kind="ExternalInput")
    wat_e = nc.dram_tensor("wat", [D, 1], f32, kind="ExternalInput")
    sidx_e = nc.dram_tensor("sidx", [nW, 128, 128], i16, kind="ExternalInput")
    dloc_e = nc.dram_tensor("dloc", [nW, 128, K], f32, kind="ExternalInput")
    nid_e = nc.dram_tensor("nid", [128, nW * 8], i16, kind="ExternalInput")
    res_e = nc.dram_tensor("res", [nW * 128, D], f32, kind="ExternalOutput")
    tbl_src = nc.dram_tensor("tblsrc", [NPAD, 128], f16, kind="Internal",
                             addr_space="Shared")

    with tile.TileContext(nc) as tc:
        with tc.tile_pool(name="c", bufs=1) as cp, \
             tc.tile_pool(name="sb", bufs=3) as sp, \
             tc.tile_pool(name="dr", bufs=1, space="DRAM") as dp:
            pp = tc.alloc_tile_pool(name="psb", bufs=1, space="PSUM")
            nc.gpsimd.load_library(library_config.mlp)
            # ---- constants
            ident_d = nc.inline_tensor(np.eye(128, dtype=np.float32),
                                       name="ident_c")
            ident16_d = nc.inline_tensor(np.eye(128, dtype=np.float16),
                                         name="ident16_c")
            iota16_d = nc.inline_tensor(
                np.tile(np.arange(128, dtype=np.float16), (128, 1)),
                name="iota16_c")
            iotaf_d = nc.inline_tensor(
                np.tile(np.arange(128, dtype=np.float32), (128, 1)),
                name="iotaf_c")
            ident = cp.tile([128, 128], f32)
            nc.sync.dma_start(out=ident[:], in_=ident_d[:])
            ident16 = cp.tile([128, 128], f16)
            nc.sync.dma_start(out=ident16[:], in_=ident16_d[:])
            iota16 = cp.tile([128, 128], f16)
            nc.sync.dma_start(out=iota16[:], in_=iota16_d[:])
            iotaf = cp.tile([128, 128], f32)
            nc.sync.dma_start(out=iotaf[:], in_=iotaf_d[:])
            ones_row16 = cp.tile([1, 128], f16)
            nc.vector.memset(ones_row16[:], 1.0)
            ones_col = cp.tile([128, 1], f32)
            nc.vector.memset(ones_col[:], 1.0)

            # ---- weight prep: rhsb [64, 66] = [W.T | 0 | W.T @ w_attn]
            wfc = cp.tile([D, D], f32)
            nc.sync.dma_start(out=wfc[:], in_=wfc_e[:])
            wat = cp.tile([D, 1], f32)
            nc.sync.dma_start(out=wat[:], in_=wat_e[:])
            wt_ps = pp.tile([D, D], f32, space="PSUM")
            nc.tensor.transpose(out=wt_ps[:], in_=wfc[:], identity=ident[:D, :D])
            v_ps = pp.tile([D, 1], f32, space="PSUM")
            nc.tensor.matmul(out=v_ps[:], lhsT=wfc[:], rhs=wat[:],
                             start=True, stop=True)
            rhsb = cp.tile([D, 66], f32)
            nc.vector.memset(rhsb[:], 0.0)
            nc.vector.tensor_copy(rhsb[:, 0:64], wt_ps[:])
            nc.vector.tensor_copy(rhsb[:, 65:66], v_ps[:])

            # ---- table build (this core's shard), fp16 256B rows
            tbl_sh = dp.tile([SHARD, 128], f16)
            tbl_dst = dp.tile([SHARD, 128], f16)
            for j in range(SHARD // 128):
                r0 = j * 128
                for (h_e, tbl, is_src) in ((hs_e, tbl_sh, True),
                                           (hd_e, tbl_dst, False)):
                    hb = sp.tile([128, D], f32, tag="bh")
                    nc.sync.dma_start(out=hb[:], in_=h_e[r0:r0 + 128, :])
                    hT_ps = pp.tile([D, 128], f32, space="PSUM", tag="bt")
                    nc.tensor.transpose(out=hT_ps[:], in_=hb[:],
                                        identity=ident[:])
                    hT = sp.tile([D, 128], f32, tag="bs")
                    nc.vector.tensor_copy(hT[:], hT_ps[:])
                    pb = pp.tile([128, 66], f32, space="PSUM", tag="bp")
                    nc.tensor.matmul(out=pb[:], lhsT=hT[:], rhs=rhsb[:],
                                     start=True, stop=True)
                    tb = sp.tile([128, 128], f16, tag="bo")
                    nc.vector.memset(tb[:], 0.0)
                    if is_src:
                        # src rows: [p(64) | 1 | q | pad]
                        nc.vector.tensor_copy(tb[:, 0:66], pb[:])
                        nc.vector.memset(tb[:, 64:65], 1.0)
                    else:
                        # dst rows: [q | p(64) | pad] (q at col 0 so the
                        # transposed gather puts it on partition 0)
                        nc.vector.tensor_copy(tb[:, 1:65], pb[:, 0:64])
                        nc.vector.tensor_copy(tb[:, 0:1], pb[:, 65:66])
                    nc.sync.dma_start(out=tbl[r0:r0 + 128, :], in_=tb[:])

            pp.release()
            pp2 = tc.alloc_tile_pool(name="psm", bufs=2, space="PSUM")

            # ---- all-gather the src table (fp16, Shared output)
            nc.gpsimd.collective_compute(
                "AllGather", mybir.AluOpType.bypass,
                replica_groups=[list(range(NC))],
                ins=[tbl_sh.opt()], outs=[tbl_src[:].opt()])

            # ---- prefetch all windows' dst rows: nrA[:, w, :] fp16
            nid_s = cp.tile([128, nW * 8], i16)
            nc.sync.dma_start(out=nid_s[:], in_=nid_e[:])
            nrA = cp.tile([128, nW, 128], f16)
            nrT = cp.tile([128, 1, nW * 128], f16)  # feature-major dst rows
            for t in range((nW + 3) // 4):  # <=512 idxs per call
                wlo = t * 4
                wcnt = min(4, nW - wlo)
                nc.gpsimd.dma_gather(
                    out_ap=nrA[:, wlo:wlo + wcnt, :], in_ap=tbl_dst[:],
                    idxs_ap=nid_s[:, wlo * 8:(wlo + wcnt) * 8],
                    num_idxs=wcnt * 128, num_idxs_reg=wcnt * 128,
                    elem_size=128)
                nc.gpsimd.dma_gather(
                    out_ap=nrT[:, :, wlo * 128:(wlo + wcnt) * 128],
                    in_ap=tbl_dst[:],
                    idxs_ap=nid_s[:, wlo * 8:(wlo + wcnt) * 8],
                    num_idxs=wcnt * 128, num_idxs_reg=wcnt * 128,
                    elem_size=128, transpose=True, queue_num=1)

            # ---- main loop
            rep_ctx = tc.For_i(0, main_repeat, 1) if main_repeat > 1 else None
            if rep_ctx is not None:
                rep_ctx.__enter__()
            for w in range(nW):
                sidx = sp.tile([128, 128], i16, tag="si")
                nc.sync.dma_start(out=sidx[:], in_=sidx_e[w])
                dloc = sp.tile([128, K], f32, tag="dl")
                nc.sync.dma_start(out=dloc[:], in_=dloc_e[w])
                # qb[p, j] = q_dst of window node j (row 0 of nrT)
                qb_ps = pp2.tile([128, 128], f32, space="PSUM", tag="qb")
                nc.tensor.matmul(out=qb_ps[:], lhsT=ones_row16[:],
                                 rhs=nrT[0:1, 0, w * 128:(w + 1) * 128],
                                 start=True, stop=True)
                qb = sp.tile([128, 128], f32, tag="qbs")
                nc.vector.tensor_copy(qb[:], qb_ps[:])

                pay = sp.tile([128, K, 128], f16, tag="pay", bufs=3)
                if ablate != "compute_only":
                    for b in range(4):
                        nc.gpsimd.dma_gather(
                            out_ap=pay[:, 4 * b:4 * b + 4, :],
                            in_ap=tbl_src[b * QS:(b + 1) * QS, :],
                            idxs_ap=sidx[:, b * 32:(b + 1) * 32],
                            num_idxs=QCAP, num_idxs_reg=QCAP,
                            elem_size=128, queue_num=b % NQUEUE)
                if ablate == "gather_only":
                    acc = pp2.tile([128, 65], f32, space="PSUM", tag="acc")
                    nc.tensor.matmul(out=acc[:], lhsT=ident16[:],
                                     rhs=pay[:, 0, 0:65], start=True,
                                     stop=True)
                else:
                    # qd[p] = q_dst[dloc[p,k]] via fused masked-sum
                    qd_all = sp.tile([128, K], f32, tag="qd")
                    for k in range(K):
                        scr = sp.tile([128, 128], f32, tag="scr", bufs=2)
                        nc.vector.scalar_tensor_tensor(
                            out=scr[:], in0=iotaf[:],
                            scalar=dloc[:, k:k + 1], in1=qb[:],
                            op0=EQ, op1=MUL,
                            accum_out=qd_all[:, k:k + 1])
                    qs32 = sp.tile([128, K], f32, tag="qs")
                    nc.vector.tensor_copy(qs32[:], pay[:, :, 65])
                    dall = sp.tile([128, K], f32, tag="da")
                    nc.vector.tensor_tensor(dall[:], qd_all[:], qs32[:],
                                            op=SUB)
                    th = sp.tile([128, K], f32, tag="th")
                    nc.scalar.activation(
                        out=th[:], in_=dall[:],
                        func=mybir.ActivationFunctionType.Tanh)
                    wall = sp.tile([128, K], f32, tag="wa")
                    nc.scalar.activation(
                        out=wall[:], in_=th[:],
                        func=mybir.ActivationFunctionType.Exp)
                    acc = pp2.tile([128, 65], f32, space="PSUM", tag="acc")
                    for k in range(K):
                        s01w = sp.tile([128, 128], f16, tag="s1", bufs=4)
                        nc.vector.tensor_scalar(
                            out=s01w[:], in0=iota16[:],
                            scalar1=dloc[:, k:k + 1],
                            scalar2=wall[:, k:k + 1], op0=EQ, op1=MUL)
                        nc.tensor.matmul(out=acc[:], lhsT=s01w[:],
                                         rhs=pay[:, k, 0:65],
                                         start=(k == 0), stop=(k == K - 1))

                # epilogue: out = elu(p_dst - swp/sw) * (sw != 0)
                z = sp.tile([128, 1], f32, tag="z")
                nc.vector.tensor_scalar(
                    out=z[:], in0=acc[:, 64:65], scalar1=0.0, scalar2=None,
                    op0=EQ)
                den = sp.tile([128, 1], f32, tag="den")
                nc.vector.tensor_tensor(den[:], acc[:, 64:65], z[:], op=ADD)
                rec = sp.tile([128, 1], f32, tag="rec")
                nc.vector.reciprocal(rec[:], den[:])
                nzm = sp.tile([128, 1], f32, tag="nzm")
                nc.vector.scalar_tensor_tensor(
                    out=nzm[:], in0=z[:], scalar=-1.0, in1=ones_col[:],
                    op0=MUL, op1=ADD)
                mean = sp.tile([128, D], f32, tag="mean")
                nc.vector.tensor_scalar(
                    out=mean[:], in0=acc[:, 0:64], scalar1=rec[:],
                    scalar2=None, op0=MUL)
                pd32 = sp.tile([128, D], f32, tag="pd")
                nc.vector.tensor_copy(pd32[:], nrA[:, w, 1:65])
                diff = sp.tile([128, D], f32, tag="diff")
                nc.vector.tensor_tensor(diff[:], pd32[:], mean[:], op=SUB)
                dm = sp.tile([128, D], f32, tag="dm")
                nc.vector.tensor_scalar(
                    out=dm[:], in0=diff[:], scalar1=nzm[:], scalar2=None,
                    op0=MUL)
                neg = sp.tile([128, D], f32, tag="neg")
                nc.vector.tensor_scalar(
                    out=neg[:], in0=dm[:], scalar1=0.0, scalar2=None,
                    op0=mybir.AluOpType.min)
                ex = sp.tile([128, D], f32, tag="ex")
                nc.scalar.activation(out=ex[:], in_=neg[:],
                                     func=mybir.ActivationFunctionType.Exp)
                pos = sp.tile([128, D], f32, tag="pos")
                nc.vector.tensor_scalar(
                    out=pos[:], in0=dm[:], scalar1=0.0, scalar2=None,
                    op0=mybir.AluOpType.max)
                res = sp.tile([128, D], f32, tag="res")
                nc.vector.scalar_tensor_tensor(
                    out=res[:], in0=ex[:], scalar=-1.0, in1=pos[:],
                    op0=ADD, op1=ADD)
                nc.sync.dma_start(out=res_e[w * 128:(w + 1) * 128, :],
                                  in_=res[:])
            if rep_ctx is not None:
                rep_ctx.__exit__(None, None, None)
            pp2.release()
    nc.compile()
    return nc


_CACHE = {}


def _get_program(nW, main_repeat, ablate=""):
    key = (nW, main_repeat, ablate)
    if key not in _CACHE:
        _CACHE[key] = _build_program(nW, main_repeat, ablate)
    return _CACHE[key]


def kernel(h_src, h_dst, W_fc, w_attn, src, dst, _main_repeat=MAIN_REPEAT,
           _return_walls=False, _ablate="", _limit_windows=0):
    from concourse.bass_utils import run_bass_kernel_spmd

    h_src = np.ascontiguousarray(np.asarray(h_src, np.float32))
    h_dst = np.ascontiguousarray(np.asarray(h_dst, np.float32))
    W_fc = np.ascontiguousarray(np.asarray(W_fc, np.float32))
    w_attn = np.ascontiguousarray(np.asarray(w_attn, np.float32)).reshape(D, 1)
    cores, nW = _prep(src, dst)
    if _limit_windows:
        nW = min(nW, _limit_windows)
        for core in cores:
            core["sidx"] = core["sidx"][:nW]
            core["dloc"] = core["dloc"][:nW]
            core["nid_all"] = core["nid_all"][:, :nW * 8]
            core["base"] = core["base"][:nW]
            core["nn"] = core["nn"][:nW]

    hp = np.zeros((NPAD, D), np.float32); hp[:N] = h_src
    hq = np.zeros((NPAD, D), np.float32); hq[:N] = h_dst

    in_maps = []
    for c, core in enumerate(cores):
        in_maps.append({
            "hs": hp[c * SHARD:(c + 1) * SHARD],
            "hd": hq[c * SHARD:(c + 1) * SHARD],
            "wfc": W_fc,
            "wat": w_attn,
            "sidx": core["sidx"],
            "dloc": core["dloc"],
            "nid": core["nid_all"],
            })
    nc = _get_program(nW, _main_repeat, _ablate)
    import time
    walls = []
    t0 = time.time()
    res = run_bass_kernel_spmd(nc, in_maps, list(range(NC)))
    walls.append(time.time() - t0)

    out = np.zeros((N, D), np.float32)
    for c, core in enumerate(cores):
        r = res.results[c]["res"].reshape(nW, 128, D)
        base, nn = core["base"], core["nn"]
        for w in range(nW):
            if nn[w] > 0:
                out[base[w]:base[w] + nn[w]] = r[w, :nn[w]]
    if _return_walls:
        return out, walls
    return out


if __name__ == "__main__":
    d = np.load("/root/problem/refdata.npz")
    out = kernel(d["h_src"], d["h_dst"], d["W_fc"], d["w_attn"],
                 d["src"], d["dst"])
    exp = d["expected"]
    rel = np.linalg.norm(out - exp) / np.linalg.norm(exp)
    print(f"rel_l2 = {rel:.3e}  maxabs = {np.abs(out - exp).max():.3e}")


# revision 11
# speedup vs baseline: 1.4415x; 1.4415x over previous
"""Trainium2 Bass kernel for nn_DiffAttention (GNN message passing), v2.

Math (per edge i: src s_i -> dst n, dst sorted):
  d_i = (h_dst[n] - h_src[s_i]) @ W_fc.T ;  e_i = tanh(d_i @ w_attn)
  alpha = segment_softmax(e, dst);  out[n] = elu(sum_i alpha_i d_i)
Since e in [-1,1], softmax needs no max-subtraction:
  out[n] = elu(p_dst[n] - (sum_i w_i p_src[s_i]) / (sum_i w_i)),
  w_i = exp(tanh(q_dst[n] - q_src[s_i])), p = h @ W_fc.T, q = p @ w_attn.

v2 device strategy (8 cores, SPMD, dst-node-range sharding):
  - fp16 node tables, 256B rows (the dma_gather granularity):
    src table [NPAD, 128] = [p(64) | 1 | q | pad62], AllGathered (Shared);
    dst table [SHARD, 128] core-local.
  - per-edge src rows fetched with gpsimd.dma_gather (custom SWDGE ucode,
    ~0.34ns/descriptor) instead of generic indirect DMA.  int16 gather
    indices only span 32K rows, so each 2048-slot window is split into 4
    node-quadrant buckets with fixed 512-slot regions.
  - per window (<=128 dst nodes):  4 gathers; one fused DVE op per
    128-edge tile produces masked q_dst broadcast + row-sum (qd) via
    accum_out; batched tanh/exp; one DVE op builds the w-scaled one-hot
    which feeds PSUM matmul accumulation of [sum w*p | sum w].
  - window dst rows for all windows are prefetch-gathered in the prologue.
Host does only index prep (windows, quadrant bucketing, int16 wrapping).
"""
import sys
sys.path.insert(0, "/opt/trn_rl_repo")
import numpy as np

N = 100000
D = 64
NC = 8
SHARD = 12544          # dst nodes owned per core; NC*SHARD = 100352
NPAD = NC * SHARD
QS = NPAD // 4         # 25088-row quadrants so gather indices fit int16
K = 16                 # 128-slot tiles per window
WE = K * 128           # 2048 edge slots per window
QCAP = 512             # slots per quadrant region (4 x 512 = 2048)
WIN_NODES = 128
MAIN_REPEAT = 1        # test.py overrides for timing
NQUEUE = 4


# ---------------------------------------------------------------- host prep
def _wrap16(vals, reps):
    # gather position i reads idxs[i % 16, i // 16]; replicate across the
    # 8 gpsimd partition groups.
    n = vals.shape[0]
    return np.tile(vals.reshape(n // 16, 16).T, (reps, 1))


def _prep_core(src, dst, c):
    n_lo = c * SHARD
    n_hi = min((c + 1) * SHARD, N)
    e_lo = np.searchsorted(dst, n_lo, side="left")
    e_hi = np.searchsorted(dst, n_hi, side="left")
    s = src[e_lo:e_hi]
    d = (dst[e_lo:e_hi] - n_lo).astype(np.int64)
    q = s // QS
    n_nodes = n_hi - n_lo
    cq = np.bincount(d * 4 + q, minlength=n_nodes * 4).reshape(n_nodes, 4)
    deg = cq.sum(1)
    estart = np.concatenate([[0], np.cumsum(deg)])
    assert cq.max() <= QCAP, f"node quadrant degree {cq.max()} > {QCAP}"

    windows = []
    n0 = 0
    while n0 < n_nodes:
        used = np.zeros(4, np.int64)
        n = n0
        while n < n_nodes and n - n0 < WIN_NODES:
            u2 = used + cq[n]
            if (u2 > QCAP).any():
                break
            used = u2
            n += 1
        windows.append((n0, n))
        n0 = n

    nW = len(windows)
    sidx = np.zeros((nW, 128, 128), np.int16)
    dloc = np.full((nW, 128, K), -1.0, np.float32)
    nidv = np.zeros((nW, 128), np.int16)
    bases, nns = [], []
    for w, (a, b) in enumerate(windows):
        elo, ehi = estart[a], estart[b]
        se, de, qe = s[elo:ehi], d[elo:ehi] - a, q[elo:ehi]
        order = np.argsort(qe, kind="stable")
        se, de, qe = se[order], de[order], qe[order]
        qcnt = np.bincount(qe, minlength=4)
        qst = np.concatenate([[0], np.cumsum(qcnt)])
        slot = np.arange(ehi - elo) - qst[qe] + qe * QCAP
        sv = np.zeros(WE, np.int16)
        dv = np.full(WE, -1.0, np.float32)
        sv[slot] = (se - qe * QS).astype(np.int16)
        dv[slot] = de.astype(np.float32)
        for bq in range(4):
            sidx[w, :, bq * 32:(bq + 1) * 32] = _wrap16(
                sv[bq * QCAP:(bq + 1) * QCAP], 8)
        dloc[w] = dv.reshape(K, 128).T
        nv = np.zeros(128, np.int16)
        nv[: b - a] = np.arange(a, b, dtype=np.int16)
        nidv[w] = nv
        bases.append(n_lo + a)
        nns.append(b - a)
    return dict(sidx=sidx, dloc=dloc, nidv=nidv,
                base=np.array(bases), nn=np.array(nns))


def _prep(src, dst):
    src = np.asarray(src, np.int64)
    dst = np.asarray(dst, np.int64)
    if np.any(np.diff(dst) < 0):
        order = np.argsort(dst, kind="stable")
        src, dst = src[order], dst[order]
    cores = [_prep_core(src, dst, c) for c in range(NC)]
    nW = max(c["sidx"].shape[0] for c in cores)
    for core in cores:
        w0 = core["sidx"].shape[0]
        pad = nW - w0
        if pad:
            core["sidx"] = np.concatenate(
                [core["sidx"], np.zeros((pad, 128, 128), np.int16)])
            core["dloc"] = np.concatenate(
                [core["dloc"], np.full((pad, 128, K), -1.0, np.float32)])
            core["nidv"] = np.concatenate(
                [core["nidv"], np.zeros((pad, 128), np.int16)])
            core["base"] = np.concatenate([core["base"], np.full(pad, N)])
            core["nn"] = np.concatenate([core["nn"], np.zeros(pad, np.int64)])
        # nid wrapped for the prologue batch gathers: global position
        # i = w*128 + j -> idx[i%16, i//16] -> col w*8 + j//16
        nid_all = np.zeros((128, nW * 8), np.int16)
        for w in range(nW):
            nid_all[:, w * 8:(w + 1) * 8] = _wrap16(core["nidv"][w], 8)
        core["nid_all"] = nid_all
    return cores, nW


# ---------------------------------------------------------------- device
def _build_program(nW, main_repeat, ablate=""):
    from concourse import bass, bacc, mybir, tile, library_config
    f32, f16 = mybir.dt.float32, mybir.dt.float16
    i16 = mybir.dt.int16
    EQ, MUL = mybir.AluOpType.is_equal, mybir.AluOpType.mult
    ADD, SUB = mybir.AluOpType.add, mybir.AluOpType.subtract

    nc = bacc.Bacc("TRN2", target_bir_lowering=False, debug=False,
                   num_devices=NC, num_swdge_queues=NQUEUE)
    hs_e = nc.dram_tensor("hs", [SHARD, D], f32, kind="ExternalInput")
    hd_e = nc.dram_tensor("hd", [SHARD, D], f32, kind="ExternalInput")
    wfc_e = nc.dram_tensor("wfc", [D, D], f32, kind="ExternalInput")
    wat_e = nc.dram_tensor("wat", [D, 1], f32, kind="ExternalInput")
    sidx_e = nc.dram_tensor("sidx", [nW, 128, 128], i16, kind="ExternalInput")
    dloc_e = nc.dram_tensor("dloc", [nW, 128, K], f32, kind="ExternalInput")
    nid_e = nc.dram_tensor("nid", [128, nW * 8], i16, kind="ExternalInput")
    res_e = nc.dram_tensor("res", [nW * 128, D], f32, kind="ExternalOutput")
    # NOTE: not addr_space="Shared" — gathers from a Shared region measure
    # ~6x slower per descriptor (worse with 8 cores contending).
    tbl_src = nc.dram_tensor("tblsrc", [NPAD, 128], f16, kind="Internal")

    with tile.TileContext(nc) as tc:
        with tc.tile_pool(name="c", bufs=1) as cp, \
             tc.tile_pool(name="sb", bufs=3) as sp, \
             tc.tile_pool(name="dr", bufs=1, space="DRAM") as dp:
            pp = tc.alloc_tile_pool(name="psb", bufs=1, space="PSUM")
            nc.gpsimd.load_library(library_config.mlp)
            # ---- constants
            ident_d = nc.inline_tensor(np.eye(128, dtype=np.float32),
                                       name="ident_c")
            ident16_d = nc.inline_tensor(np.eye(128, dtype=np.float16),
                                         name="ident16_c")
            iota16_d = nc.inline_tensor(
                np.tile(np.arange(128, dtype=np.float16), (128, 1)),
                name="iota16_c")
            iotaf_d = nc.inline_tensor(
                np.tile(np.arange(128, dtype=np.float32), (128, 1)),
                name="iotaf_c")
            ident = cp.tile([128, 128], f32)
            nc.sync.dma_start(out=ident[:], in_=ident_d[:])
            ident16 = cp.tile([128, 128], f16)
            nc.sync.dma_start(out=ident16[:], in_=ident16_d[:])
            iota16 = cp.tile([128, 128], f16)
            nc.sync.dma_start(out=iota16[:], in_=iota16_d[:])
            iotaf = cp.tile([128, 128], f32)
            nc.sync.dma_start(out=iotaf[:], in_=iotaf_d[:])
            ones_row16 = cp.tile([1, 128], f16)
            nc.vector.memset(ones_row16[:], 1.0)
            ones_col = cp.tile([128, 1], f32)
            nc.vector.memset(ones_col[:], 1.0)

            # ---- weight prep: rhsb [64, 66] = [W.T | 0 | W.T @ w_attn]
            wfc = cp.tile([D, D], f32)
            nc.sync.dma_start(out=wfc[:], in_=wfc_e[:])
            wat = cp.tile([D, 1], f32)
            nc.sync.dma_start(out=wat[:], in_=wat_e[:])
            wt_ps = pp.tile([D, D], f32, space="PSUM")
            nc.tensor.transpose(out=wt_ps[:], in_=wfc[:], identity=ident[:D, :D])
            v_ps = pp.tile([D, 1], f32, space="PSUM")
            nc.tensor.matmul(out=v_ps[:], lhsT=wfc[:], rhs=wat[:],
                             start=True, stop=True)
            rhsb = cp.tile([D, 66], f32)
            nc.vector.memset(rhsb[:], 0.0)
            nc.vector.tensor_copy(rhsb[:, 0:64], wt_ps[:])
            nc.vector.tensor_copy(rhsb[:, 65:66], v_ps[:])

            # ---- table build (this core's shard), fp16 256B rows
            tbl_sh = dp.tile([SHARD, 128], f16)
            tbl_dst = dp.tile([SHARD, 128], f16)
            for j in range(SHARD // 128):
                r0 = j * 128
                for (h_e, tbl, is_src) in ((hs_e, tbl_sh, True),
                                           (hd_e, tbl_dst, False)):
                    hb = sp.tile([128, D], f32, tag="bh")
                    nc.sync.dma_start(out=hb[:], in_=h_e[r0:r0 + 128, :])
                    hT_ps = pp.tile([D, 128], f32, space="PSUM", tag="bt")
                    nc.tensor.transpose(out=hT_ps[:], in_=hb[:],
                                        identity=ident[:])
                    hT = sp.tile([D, 128], f32, tag="bs")
                    nc.vector.tensor_copy(hT[:], hT_ps[:])
                    pb = pp.tile([128, 66], f32, space="PSUM", tag="bp")
                    nc.tensor.matmul(out=pb[:], lhsT=hT[:], rhs=rhsb[:],
                                     start=True, stop=True)
                    tb = sp.tile([128, 128], f16, tag="bo")
                    nc.vector.memset(tb[:], 0.0)
                    if is_src:
                        # src rows: [p(64) | 1 | q | pad]
                        nc.vector.tensor_copy(tb[:, 0:66], pb[:])
                        nc.vector.memset(tb[:, 64:65], 1.0)
                    else:
                        # dst rows: [q | p(64) | pad] (q at col 0 so the
                        # transposed gather puts it on partition 0)
                        nc.vector.tensor_copy(tb[:, 1:65], pb[:, 0:64])
                        nc.vector.tensor_copy(tb[:, 0:1], pb[:, 65:66])
                    nc.sync.dma_start(out=tbl[r0:r0 + 128, :], in_=tb[:])

            pp.release()
            pp2 = tc.alloc_tile_pool(name="psm", bufs=2, space="PSUM")

            # ---- all-gather the src table (fp16, Shared output)
            nc.gpsimd.collective_compute(
                "AllGather", mybir.AluOpType.bypass,
                replica_groups=[list(range(NC))],
                ins=[tbl_sh.opt()], outs=[tbl_src[:].opt()])

            # ---- prefetch all windows' dst rows: nrA[:, w, :] fp16
            nid_s = cp.tile([128, nW * 8], i16)
            nc.sync.dma_start(out=nid_s[:], in_=nid_e[:])
            nrA = cp.tile([128, nW, 128], f16)
            nrT = cp.tile([128, 1, nW * 128], f16)  # feature-major dst rows
            for t in range((nW + 3) // 4):  # <=512 idxs per call
                wlo = t * 4
                wcnt = min(4, nW - wlo)
                nc.gpsimd.dma_gather(
                    out_ap=nrA[:, wlo:wlo + wcnt, :], in_ap=tbl_dst[:],
                    idxs_ap=nid_s[:, wlo * 8:(wlo + wcnt) * 8],
                    num_idxs=wcnt * 128, num_idxs_reg=wcnt * 128,
                    elem_size=128)
                nc.gpsimd.dma_gather(
                    out_ap=nrT[:, :, wlo * 128:(wlo + wcnt) * 128],
                    in_ap=tbl_dst[:],
                    idxs_ap=nid_s[:, wlo * 8:(wlo + wcnt) * 8],
                    num_idxs=wcnt * 128, num_idxs_reg=wcnt * 128,
                    elem_size=128, transpose=True, queue_num=1)

            # ---- main loop
            rep_ctx = tc.For_i(0, main_repeat, 1) if main_repeat > 1 else None
            if rep_ctx is not None:
                rep_ctx.__enter__()
            for w in range(nW):
                sidx = sp.tile([128, 128], i16, tag="si")
                nc.sync.dma_start(out=sidx[:], in_=sidx_e[w])
                dloc = sp.tile([128, K], f32, tag="dl")
                nc.sync.dma_start(out=dloc[:], in_=dloc_e[w])
                # qb[p, j] = q_dst of window node j (row 0 of nrT)
                qb_ps = pp2.tile([128, 128], f32, space="PSUM", tag="qb")
                nc.tensor.matmul(out=qb_ps[:], lhsT=ones_row16[:],
                                 rhs=nrT[0:1, 0, w * 128:(w + 1) * 128],
                                 start=True, stop=True)
                qb = sp.tile([128, 128], f32, tag="qbs")
                nc.vector.tensor_copy(qb[:], qb_ps[:])

                pay = sp.tile([128, K, 128], f16, tag="pay", bufs=3)
                if ablate != "compute_only":
                    for b in range(4):
                        nc.gpsimd.dma_gather(
                            out_ap=pay[:, 4 * b:4 * b + 4, :],
                            in_ap=tbl_src[b * QS:(b + 1) * QS, :],
                            idxs_ap=sidx[:, b * 32:(b + 1) * 32],
                            num_idxs=QCAP, num_idxs_reg=QCAP,
                            elem_size=128, queue_num=b % NQUEUE)
                if ablate == "gather_only":
                    acc = pp2.tile([128, 65], f32, space="PSUM", tag="acc")
                    nc.tensor.matmul(out=acc[:], lhsT=ident16[:],
                                     rhs=pay[:, 0, 0:65], start=True,
                                     stop=True)
                else:
                    # qd[p] = q_dst[dloc[p,k]] via fused masked-sum
                    qd_all = sp.tile([128, K], f32, tag="qd")
                    for k in range(K):
                        scr = sp.tile([128, 128], f32, tag="scr", bufs=2)
                        nc.vector.scalar_tensor_tensor(
                            out=scr[:], in0=iotaf[:],
                            scalar=dloc[:, k:k + 1], in1=qb[:],
                            op0=EQ, op1=MUL,
                            accum_out=qd_all[:, k:k + 1])
                    qs32 = sp.tile([128, K], f32, tag="qs")
                    nc.vector.tensor_copy(qs32[:], pay[:, :, 65])
                    dall = sp.tile([128, K], f32, tag="da")
                    nc.vector.tensor_tensor(dall[:], qd_all[:], qs32[:],
                                            op=SUB)
                    th = sp.tile([128, K], f32, tag="th")
                    nc.scalar.activation(
                        out=th[:], in_=dall[:],
                        func=mybir.ActivationFunctionType.Tanh)
                    wall = sp.tile([128, K], f32, tag="wa")
                    nc.scalar.activation(
                        out=wall[:], in_=th[:],
                        func=mybir.ActivationFunctionType.Exp)
                    acc = pp2.tile([128, 65], f32, space="PSUM", tag="acc")
                    for k in range(K):
                        s01w = sp.tile([128, 128], f16, tag="s1", bufs=4)
                        nc.vector.tensor_scalar(
                            out=s01w[:], in0=iota16[:],
                            scalar1=dloc[:, k:k + 1],
                            scalar2=wall[:, k:k + 1], op0=EQ, op1=MUL)
                        nc.tensor.matmul(out=acc[:], lhsT=s01w[:],
                                         rhs=pay[:, k, 0:65],
                                         start=(k == 0), stop=(k == K - 1))

                # epilogue: out = elu(p_dst - swp/sw) * (sw != 0)
                z = sp.tile([128, 1], f32, tag="z")
                nc.vector.tensor_scalar(
                    out=z[:], in0=acc[:, 64:65], scalar1=0.0, scalar2=None,
                    op0=EQ)
                den = sp.tile([128, 1], f32, tag="den")
                nc.vector.tensor_tensor(den[:], acc[:, 64:65], z[:], op=ADD)
                rec = sp.tile([128, 1], f32, tag="rec")
                nc.vector.reciprocal(rec[:], den[:])
                nzm = sp.tile([128, 1], f32, tag="nzm")
                nc.vector.scalar_tensor_tensor(
                    out=nzm[:], in0=z[:], scalar=-1.0, in1=ones_col[:],
                    op0=MUL, op1=ADD)
                mean = sp.tile([128, D], f32, tag="mean")
                nc.vector.tensor_scalar(
                    out=mean[:], in0=acc[:, 0:64], scalar1=rec[:],
                    scalar2=None, op0=MUL)
                pd32 = sp.tile([128, D], f32, tag="pd")
                nc.vector.tensor_copy(pd32[:], nrA[:, w, 1:65])
                diff = sp.tile([128, D], f32, tag="diff")
                nc.vector.tensor_tensor(diff[:], pd32[:], mean[:], op=SUB)
                dm = sp.tile([128, D], f32, tag="dm")
                nc.vector.tensor_scalar(
                    out=dm[:], in0=diff[:], scalar1=nzm[:], scalar2=None,
                    op0=MUL)
                neg = sp.tile([128, D], f32, tag="neg")
                nc.vector.tensor_scalar(
                    out=neg[:], in0=dm[:], scalar1=0.0, scalar2=None,
                    op0=mybir.AluOpType.min)
                ex = sp.tile([128, D], f32, tag="ex")
                nc.scalar.activation(out=ex[:], in_=neg[:],
                                     func=mybir.ActivationFunctionType.Exp)
                pos = sp.tile([128, D], f32, tag="pos")
                nc.vector.tensor_scalar(
                    out=pos[:], in0=dm[:], scalar1=0.0, scalar2=None,
                    op0=mybir.AluOpType.max)
                res = sp.tile([128, D], f32, tag="res")
                nc.vector.scalar_tensor_tensor(
                    out=res[:], in0=ex[:], scalar=-1.0, in1=pos[:],
                    op0=ADD, op1=ADD)
                nc.sync.dma_start(out=res_e[w * 128:(w + 1) * 128, :],
                                  in_=res[:])
            if rep_ctx is not None:
                rep_ctx.__exit__(None, None, None)
            pp2.release()
    nc.compile()
    return nc


_CACHE = {}


def _get_program(nW, main_repeat, ablate=""):
    key = (nW, main_repeat, ablate)
    if key not in _CACHE:
        _CACHE[key] = _build_program(nW, main_repeat, ablate)
    return _CACHE[key]


def kernel(h_src, h_dst, W_fc, w_attn, src, dst, _main_repeat=MAIN_REPEAT,
           _return_walls=False, _ablate="", _limit_windows=0):
    from concourse.bass_utils import run_bass_kernel_spmd

    h_src = np.ascontiguousarray(np.asarray(h_src, np.float32))
    h_dst = np.ascontiguousarray(np.asarray(h_dst, np.float32))
    W_fc = np.ascontiguousarray(np.asarray(W_fc, np.float32))
    w_attn = np.ascontiguousarray(np.asarray(w_attn, np.float32)).reshape(D, 1)
    cores, nW = _prep(src, dst)
    if _limit_windows:
        nW = min(nW, _limit_windows)
        for core in cores:
            core["sidx"] = core["sidx"][:nW]
            core["dloc"] = core["dloc"][:nW]
            core["nid_all"] = core["nid_all"][:, :nW * 8]
            core["base"] = core["base"][:nW]
            core["nn"] = core["nn"][:nW]

    hp = np.zeros((NPAD, D), np.float32); hp[:N] = h_src
    hq = np.zeros((NPAD, D), np.float32); hq[:N] = h_dst

    in_maps = []
    for c, core in enumerate(cores):
        in_maps.append({
            "hs": hp[c * SHARD:(c + 1) * SHARD],
            "hd": hq[c * SHARD:(c + 1) * SHARD],
            "wfc": W_fc,
            "wat": w_attn,
            "sidx": core["sidx"],
            "dloc": core["dloc"],
            "nid": core["nid_all"],
            })
    nc = _get_program(nW, _main_repeat, _ablate)
    import time
    walls = []
    t0 = time.time()
    res = run_bass_kernel_spmd(nc, in_maps, list(range(NC)))
    walls.append(time.time() - t0)

    out = np.zeros((N, D), np.float32)
    for c, core in enumerate(cores):
        r = res.results[c]["res"].reshape(nW, 128, D)
        base, nn = core["base"], core["nn"]
        for w in range(nW):
            if nn[w] > 0:
                out[base[w]:base[w] + nn[w]] = r[w, :nn[w]]
    if _return_walls:
        return out, walls
    return out


if __name__ == "__main__":
    d = np.load("/root/problem/refdata.npz")
    out = kernel(d["h_src"], d["h_dst"], d["W_fc"], d["w_attn"],
                 d["src"], d["dst"])
    exp = d["expected"]
    rel = np.linalg.norm(out - exp) / np.linalg.norm(exp)
    print(f"rel_l2 = {rel:.3e}  maxabs = {np.abs(out - exp).max():.3e}")


# revision 15
# speedup vs baseline: 2.0123x; 1.3960x over previous
"""Trainium2 Bass kernel for nn_DiffAttention (GNN message passing), v2.

Math (per edge i: src s_i -> dst n, dst sorted):
  d_i = (h_dst[n] - h_src[s_i]) @ W_fc.T ;  e_i = tanh(d_i @ w_attn)
  alpha = segment_softmax(e, dst);  out[n] = elu(sum_i alpha_i d_i)
Since e in [-1,1], softmax needs no max-subtraction:
  out[n] = elu(p_dst[n] - (sum_i w_i p_src[s_i]) / (sum_i w_i)),
  w_i = exp(tanh(q_dst[n] - q_src[s_i])), p = h @ W_fc.T, q = p @ w_attn.

v2 device strategy (8 cores, SPMD, dst-node-range sharding):
  - fp16 node tables, 256B rows (the dma_gather granularity):
    src table [NPAD, 128] = [p(64) | 1 | q | pad62], AllGathered (Shared);
    dst table [SHARD, 128] core-local.
  - per-edge src rows fetched with gpsimd.dma_gather (custom SWDGE ucode,
    ~0.34ns/descriptor) instead of generic indirect DMA.  int16 gather
    indices only span 32K rows, so each 2048-slot window is split into 4
    node-quadrant buckets with fixed 512-slot regions.
  - per window (<=128 dst nodes):  4 gathers; one fused DVE op per
    128-edge tile produces masked q_dst broadcast + row-sum (qd) via
    accum_out; batched tanh/exp; one DVE op builds the w-scaled one-hot
    which feeds PSUM matmul accumulation of [sum w*p | sum w].
  - window dst rows for all windows are prefetch-gathered in the prologue.
Host does only index prep (windows, quadrant bucketing, int16 wrapping).
"""
import sys
sys.path.insert(0, "/opt/trn_rl_repo")
import numpy as np

N = 100000
D = 64
NC = 8
SHARD = 12544          # dst nodes owned per core; NC*SHARD = 100352
NPAD = NC * SHARD
QS = NPAD // 4
K = 16                 # 128-slot tiles per window
WE = K * 128           # 2048 edge slots per window
DUMMY = N              # zero table row for padded edge slots
WIN_NODES = 128
MAIN_REPEAT = 1        # test.py overrides for timing
NQUEUE = 4


# ---------------------------------------------------------------- host prep
def _wrap16(vals, reps):
    # gather position i reads idxs[i % 16, i // 16]; replicate across the
    # 8 gpsimd partition groups.
    n = vals.shape[0]
    return np.tile(vals.reshape(n // 16, 16).T, (reps, 1))


def _prep_core(src, dst, c):
    n_lo = c * SHARD
    n_hi = min((c + 1) * SHARD, N)
    e_lo = np.searchsorted(dst, n_lo, side="left")
    e_hi = np.searchsorted(dst, n_hi, side="left")
    s = src[e_lo:e_hi]
    d = (dst[e_lo:e_hi] - n_lo).astype(np.int64)
    n_nodes = n_hi - n_lo
    deg = np.bincount(d, minlength=n_nodes)
    assert deg.max() <= WE, f"node degree {deg.max()} > {WE}"
    estart = np.concatenate([[0], np.cumsum(deg)])

    windows = []
    n0 = 0
    while n0 < n_nodes:
        n_end = min(n0 + WIN_NODES, n_nodes)
        while estart[n_end] - estart[n0] > WE:
            n_end -= 1
        windows.append((n0, n_end))
        n0 = n_end

    nW = len(windows)
    sidx = np.zeros((nW, 128, K), np.int32)
    dloc = np.full((nW, 128, K), -1.0, np.float32)
    nidv = np.zeros((nW, 128), np.int16)
    bases, nns = [], []
    for w, (a, b) in enumerate(windows):
        elo, ehi = estart[a], estart[b]
        ecnt = ehi - elo
        sv = np.full(WE, DUMMY, np.int32)
        dv = np.full(WE, -1.0, np.float32)
        sv[:ecnt] = s[elo:ehi]
        dv[:ecnt] = (d[elo:ehi] - a).astype(np.float32)
        sidx[w] = sv.reshape(K, 128).T
        dloc[w] = dv.reshape(K, 128).T
        nv = np.zeros(128, np.int16)
        nv[: b - a] = np.arange(a, b, dtype=np.int16)
        nidv[w] = nv
        bases.append(n_lo + a)
        nns.append(b - a)
    return dict(sidx=sidx, dloc=dloc, nidv=nidv,
                base=np.array(bases), nn=np.array(nns))


def _prep(src, dst):
    src = np.asarray(src, np.int64)
    dst = np.asarray(dst, np.int64)
    if np.any(np.diff(dst) < 0):
        order = np.argsort(dst, kind="stable")
        src, dst = src[order], dst[order]
    cores = [_prep_core(src, dst, c) for c in range(NC)]
    nW = max(c["sidx"].shape[0] for c in cores)
    for core in cores:
        w0 = core["sidx"].shape[0]
        pad = nW - w0
        if pad:
            core["sidx"] = np.concatenate(
                [core["sidx"], np.full((pad, 128, K), DUMMY, np.int32)])
            core["dloc"] = np.concatenate(
                [core["dloc"], np.full((pad, 128, K), -1.0, np.float32)])
            core["nidv"] = np.concatenate(
                [core["nidv"], np.zeros((pad, 128), np.int16)])
            core["base"] = np.concatenate([core["base"], np.full(pad, N)])
            core["nn"] = np.concatenate([core["nn"], np.zeros(pad, np.int64)])
        # nid wrapped for the prologue batch gathers: global position
        # i = w*128 + j -> idx[i%16, i//16] -> col w*8 + j//16
        nid_all = np.zeros((128, nW * 8), np.int16)
        for w in range(nW):
            nid_all[:, w * 8:(w + 1) * 8] = _wrap16(core["nidv"][w], 8)
        core["nid_all"] = nid_all
    return cores, nW


# ---------------------------------------------------------------- device
def _build_program(nW, main_repeat, ablate=""):
    from concourse import bass, bacc, mybir, tile, library_config
    ab_gather_only = ablate.startswith("gather_only")
    ab_compute_only = ablate.startswith("compute_only")
    g_queue = (lambda b: 0) if "_q0" in ablate else (lambda b: b % NQUEUE)
    g_sp = "_sp0" not in ablate
    f32, f16 = mybir.dt.float32, mybir.dt.float16
    i16, i32 = mybir.dt.int16, mybir.dt.int32
    EQ, MUL = mybir.AluOpType.is_equal, mybir.AluOpType.mult
    ADD, SUB = mybir.AluOpType.add, mybir.AluOpType.subtract

    nc = bacc.Bacc("TRN2", target_bir_lowering=False, debug=False,
                   num_devices=NC, num_swdge_queues=NQUEUE)
    hs_e = nc.dram_tensor("hs", [SHARD, D], f32, kind="ExternalInput")
    hd_e = nc.dram_tensor("hd", [SHARD, D], f32, kind="ExternalInput")
    wfc_e = nc.dram_tensor("wfc", [D, D], f32, kind="ExternalInput")
    wat_e = nc.dram_tensor("wat", [D, 1], f32, kind="ExternalInput")
    sidx_e = nc.dram_tensor("sidx", [nW, 128, K], i32, kind="ExternalInput")
    dloc_e = nc.dram_tensor("dloc", [nW, 128, K], f32, kind="ExternalInput")
    nid_e = nc.dram_tensor("nid", [128, nW * 8], i16, kind="ExternalInput")
    res_e = nc.dram_tensor("res", [nW * 128, D], f32, kind="ExternalOutput")
    # NOTE: not addr_space="Shared" — gathers from a Shared region measure
    # ~6x slower per descriptor (worse with 8 cores contending).
    tbl_src = nc.dram_tensor("tblsrc", [NPAD, 128], f16, kind="Internal")

    with tile.TileContext(nc) as tc:
        with tc.tile_pool(name="c", bufs=1) as cp, \
             tc.tile_pool(name="sb", bufs=3) as sp, \
             tc.tile_pool(name="dr", bufs=1, space="DRAM") as dp:
            pp = tc.alloc_tile_pool(name="psb", bufs=1, space="PSUM")
            nc.gpsimd.load_library(library_config.mlp)
            # ---- constants
            ident_d = nc.inline_tensor(np.eye(128, dtype=np.float32),
                                       name="ident_c")
            ident16_d = nc.inline_tensor(np.eye(128, dtype=np.float16),
                                         name="ident16_c")
            iota16_d = nc.inline_tensor(
                np.tile(np.arange(128, dtype=np.float16), (128, 1)),
                name="iota16_c")
            iotaf_d = nc.inline_tensor(
                np.tile(np.arange(128, dtype=np.float32), (128, 1)),
                name="iotaf_c")
            ident = cp.tile([128, 128], f32)
            nc.sync.dma_start(out=ident[:], in_=ident_d[:])
            ident16 = cp.tile([128, 128], f16)
            nc.sync.dma_start(out=ident16[:], in_=ident16_d[:])
            iota16 = cp.tile([128, 128], f16)
            nc.sync.dma_start(out=iota16[:], in_=iota16_d[:])
            iotaf = cp.tile([128, 128], f32)
            nc.sync.dma_start(out=iotaf[:], in_=iotaf_d[:])
            ones_row16 = cp.tile([1, 128], f16)
            nc.vector.memset(ones_row16[:], 1.0)
            ones_col = cp.tile([128, 1], f32)
            nc.vector.memset(ones_col[:], 1.0)

            # ---- weight prep: rhsb [64, 66] = [W.T | 0 | W.T @ w_attn]
            wfc = cp.tile([D, D], f32)
            nc.sync.dma_start(out=wfc[:], in_=wfc_e[:])
            wat = cp.tile([D, 1], f32)
            nc.sync.dma_start(out=wat[:], in_=wat_e[:])
            wt_ps = pp.tile([D, D], f32, space="PSUM")
            nc.tensor.transpose(out=wt_ps[:], in_=wfc[:], identity=ident[:D, :D])
            v_ps = pp.tile([D, 1], f32, space="PSUM")
            nc.tensor.matmul(out=v_ps[:], lhsT=wfc[:], rhs=wat[:],
                             start=True, stop=True)
            rhsb = cp.tile([D, 66], f32)
            nc.vector.memset(rhsb[:], 0.0)
            nc.vector.tensor_copy(rhsb[:, 0:64], wt_ps[:])
            nc.vector.tensor_copy(rhsb[:, 65:66], v_ps[:])

            # ---- table build (this core's shard), fp16 256B rows
            tbl_sh = dp.tile([SHARD, 128], f16)
            tbl_dst = dp.tile([SHARD, 128], f16)
            for j in range(SHARD // 128):
                r0 = j * 128
                for (h_e, tbl, is_src) in ((hs_e, tbl_sh, True),
                                           (hd_e, tbl_dst, False)):
                    hb = sp.tile([128, D], f32, tag="bh")
                    nc.sync.dma_start(out=hb[:], in_=h_e[r0:r0 + 128, :])
                    hT_ps = pp.tile([D, 128], f32, space="PSUM", tag="bt")
                    nc.tensor.transpose(out=hT_ps[:], in_=hb[:],
                                        identity=ident[:])
                    hT = sp.tile([D, 128], f32, tag="bs")
                    nc.vector.tensor_copy(hT[:], hT_ps[:])
                    pb = pp.tile([128, 66], f32, space="PSUM", tag="bp")
                    nc.tensor.matmul(out=pb[:], lhsT=hT[:], rhs=rhsb[:],
                                     start=True, stop=True)
                    tb = sp.tile([128, 128], f16, tag="bo")
                    nc.vector.memset(tb[:], 0.0)
                    if is_src:
                        # src rows: [p(64) | 1 | q | pad]
                        nc.vector.tensor_copy(tb[:, 0:66], pb[:])
                        nc.vector.memset(tb[:, 64:65], 1.0)
                    else:
                        # dst rows: [q | p(64) | pad] (q at col 0 so the
                        # transposed gather puts it on partition 0)
                        nc.vector.tensor_copy(tb[:, 1:65], pb[:, 0:64])
                        nc.vector.tensor_copy(tb[:, 0:1], pb[:, 65:66])
                    nc.sync.dma_start(out=tbl[r0:r0 + 128, :], in_=tb[:])

            pp.release()
            pp2 = tc.alloc_tile_pool(name="psm", bufs=2, space="PSUM")

            # ---- all-gather the src table (fp16, Shared output)
            nc.gpsimd.collective_compute(
                "AllGather", mybir.AluOpType.bypass,
                replica_groups=[list(range(NC))],
                ins=[tbl_sh.opt()], outs=[tbl_src[:].opt()])

            # ---- prefetch all windows' dst rows: nrA[:, w, :] fp16
            nid_s = cp.tile([128, nW * 8], i16)
            nc.sync.dma_start(out=nid_s[:], in_=nid_e[:])
            nrA = cp.tile([128, nW, 128], f16)
            nrT = cp.tile([128, 1, nW * 128], f16)  # feature-major dst rows
            for t in range((nW + 3) // 4):  # <=512 idxs per call
                wlo = t * 4
                wcnt = min(4, nW - wlo)
                nc.gpsimd.dma_gather(
                    out_ap=nrA[:, wlo:wlo + wcnt, :], in_ap=tbl_dst[:],
                    idxs_ap=nid_s[:, wlo * 8:(wlo + wcnt) * 8],
                    num_idxs=wcnt * 128, num_idxs_reg=wcnt * 128,
                    elem_size=128)
                nc.gpsimd.dma_gather(
                    out_ap=nrT[:, :, wlo * 128:(wlo + wcnt) * 128],
                    in_ap=tbl_dst[:],
                    idxs_ap=nid_s[:, wlo * 8:(wlo + wcnt) * 8],
                    num_idxs=wcnt * 128, num_idxs_reg=wcnt * 128,
                    elem_size=128, transpose=True, queue_num=1)

            # ---- main loop
            rep_ctx = tc.For_i(0, main_repeat, 1) if main_repeat > 1 else None
            if rep_ctx is not None:
                rep_ctx.__enter__()
            for w in range(nW):
                sidx = sp.tile([128, K], i32, tag="si")
                nc.sync.dma_start(out=sidx[:], in_=sidx_e[w])
                dloc = sp.tile([128, K], f32, tag="dl")
                nc.sync.dma_start(out=dloc[:], in_=dloc_e[w])
                # qb[p, j] = q_dst of window node j (row 0 of nrT)
                qb_ps = pp2.tile([128, 128], f32, space="PSUM", tag="qb")
                nc.tensor.matmul(out=qb_ps[:], lhsT=ones_row16[:],
                                 rhs=nrT[0:1, 0, w * 128:(w + 1) * 128],
                                 start=True, stop=True)
                qb = sp.tile([128, 128], f16, tag="qbs")
                nc.vector.tensor_copy(qb[:], qb_ps[:])

                pay = sp.tile([128, K, 128], f16, tag="pay", bufs=3)
                if ab_compute_only:
                    nc.vector.memset(pay[:, 0, 0:1], 0.0)
                if not ab_compute_only:
                    for k in range(K):
                        nc.gpsimd.indirect_dma_start(
                            out=pay[:, k, :], out_offset=None,
                            in_=tbl_src[:],
                            in_offset=bass.IndirectOffsetOnAxis(
                                ap=sidx[:, k:k + 1], axis=0))
                if ab_gather_only:
                    acc = pp2.tile([128, 65], f32, space="PSUM", tag="acc")
                    nc.tensor.matmul(out=acc[:], lhsT=ident16[:],
                                     rhs=pay[:, 0, 0:65], start=True,
                                     stop=True)
                else:
                    # qd[p] = q_dst[dloc[p,k]]: one-hot, mask, row-sum
                    # (no accum_out — it measures ~12.7us/op on this hw)
                    qd_all = sp.tile([128, K], f32, tag="qd")
                    s01s = []
                    for k in range(K):
                        s01 = sp.tile([128, 128], f16, tag="s01",
                                      bufs=K + 2)
                        nc.vector.tensor_scalar(
                            out=s01[:], in0=iota16[:],
                            scalar1=dloc[:, k:k + 1], scalar2=None, op0=EQ)
                        s01s.append(s01)
                        scr = sp.tile([128, 128], f16, tag="scr", bufs=2)
                        nc.vector.tensor_tensor(scr[:], s01[:], qb[:],
                                                op=MUL)
                        nc.vector.tensor_reduce(
                            out=qd_all[:, k:k + 1], in_=scr[:],
                            axis=mybir.AxisListType.X, op=ADD)
                    qs32 = sp.tile([128, K], f32, tag="qs")
                    nc.vector.tensor_copy(qs32[:], pay[:, :, 65])
                    dall = sp.tile([128, K], f32, tag="da")
                    nc.vector.tensor_tensor(dall[:], qd_all[:], qs32[:],
                                            op=SUB)
                    th = sp.tile([128, K], f32, tag="th")
                    nc.scalar.activation(
                        out=th[:], in_=dall[:],
                        func=mybir.ActivationFunctionType.Tanh)
                    wall = sp.tile([128, K], f32, tag="wa")
                    nc.scalar.activation(
                        out=wall[:], in_=th[:],
                        func=mybir.ActivationFunctionType.Exp)
                    acc = pp2.tile([128, 65], f32, space="PSUM", tag="acc")
                    for k in range(K):
                        sc = sp.tile([128, 65], f16, tag="sc", bufs=4)
                        nc.scalar.activation(
                            out=sc[:], in_=pay[:, k, 0:65],
                            func=mybir.ActivationFunctionType.Copy,
                            scale=wall[:, k:k + 1])
                        nc.tensor.matmul(out=acc[:], lhsT=s01s[k][:],
                                         rhs=sc[:],
                                         start=(k == 0), stop=(k == K - 1))

                # epilogue: out = elu(p_dst - swp/sw) * (sw != 0)
                z = sp.tile([128, 1], f32, tag="z")
                nc.vector.tensor_scalar(
                    out=z[:], in0=acc[:, 64:65], scalar1=0.0, scalar2=None,
                    op0=EQ)
                den = sp.tile([128, 1], f32, tag="den")
                nc.vector.tensor_tensor(den[:], acc[:, 64:65], z[:], op=ADD)
                rec = sp.tile([128, 1], f32, tag="rec")
                nc.vector.reciprocal(rec[:], den[:])
                nzm = sp.tile([128, 1], f32, tag="nzm")
                nc.vector.scalar_tensor_tensor(
                    out=nzm[:], in0=z[:], scalar=-1.0, in1=ones_col[:],
                    op0=MUL, op1=ADD)
                mean = sp.tile([128, D], f32, tag="mean")
                nc.vector.tensor_scalar(
                    out=mean[:], in0=acc[:, 0:64], scalar1=rec[:],
                    scalar2=None, op0=MUL)
                pd32 = sp.tile([128, D], f32, tag="pd")
                nc.vector.tensor_copy(pd32[:], nrA[:, w, 1:65])
                diff = sp.tile([128, D], f32, tag="diff")
                nc.vector.tensor_tensor(diff[:], pd32[:], mean[:], op=SUB)
                dm = sp.tile([128, D], f32, tag="dm")
                nc.vector.tensor_scalar(
                    out=dm[:], in0=diff[:], scalar1=nzm[:], scalar2=None,
                    op0=MUL)
                neg = sp.tile([128, D], f32, tag="neg")
                nc.vector.tensor_scalar(
                    out=neg[:], in0=dm[:], scalar1=0.0, scalar2=None,
                    op0=mybir.AluOpType.min)
                ex = sp.tile([128, D], f32, tag="ex")
                nc.scalar.activation(out=ex[:], in_=neg[:],
                                     func=mybir.ActivationFunctionType.Exp)
                pos = sp.tile([128, D], f32, tag="pos")
                nc.vector.tensor_scalar(
                    out=pos[:], in0=dm[:], scalar1=0.0, scalar2=None,
                    op0=mybir.AluOpType.max)
                res = sp.tile([128, D], f32, tag="res")
                nc.vector.scalar_tensor_tensor(
                    out=res[:], in0=ex[:], scalar=-1.0, in1=pos[:],
                    op0=ADD, op1=ADD)
                nc.sync.dma_start(out=res_e[w * 128:(w + 1) * 128, :],
                                  in_=res[:])
            if rep_ctx is not None:
                rep_ctx.__exit__(None, None, None)
            pp2.release()
    nc.compile()
    return nc


_CACHE = {}


def _get_program(nW, main_repeat, ablate=""):
    key = (nW, main_repeat, ablate)
    if key not in _CACHE:
        _CACHE[key] = _build_program(nW, main_repeat, ablate)
    return _CACHE[key]


def kernel(h_src, h_dst, W_fc, w_attn, src, dst, _main_repeat=MAIN_REPEAT,
           _return_walls=False, _ablate="", _limit_windows=0):
    from concourse.bass_utils import run_bass_kernel_spmd

    h_src = np.ascontiguousarray(np.asarray(h_src, np.float32))
    h_dst = np.ascontiguousarray(np.asarray(h_dst, np.float32))
    W_fc = np.ascontiguousarray(np.asarray(W_fc, np.float32))
    w_attn = np.ascontiguousarray(np.asarray(w_attn, np.float32)).reshape(D, 1)
    cores, nW = _prep(src, dst)
    if _limit_windows:
        nW = min(nW, _limit_windows)
        for core in cores:
            core["sidx"] = core["sidx"][:nW]
            core["dloc"] = core["dloc"][:nW]
            core["nid_all"] = core["nid_all"][:, :nW * 8]
            core["base"] = core["base"][:nW]
            core["nn"] = core["nn"][:nW]

    hp = np.zeros((NPAD, D), np.float32); hp[:N] = h_src
    hq = np.zeros((NPAD, D), np.float32); hq[:N] = h_dst

    in_maps = []
    for c, core in enumerate(cores):
        in_maps.append({
            "hs": hp[c * SHARD:(c + 1) * SHARD],
            "hd": hq[c * SHARD:(c + 1) * SHARD],
            "wfc": W_fc,
            "wat": w_attn,
            "sidx": core["sidx"],
            "dloc": core["dloc"],
            "nid": core["nid_all"],
            })
    nc = _get_program(nW, _main_repeat, _ablate)
    import time
    walls = []
    t0 = time.time()
    res = run_bass_kernel_spmd(nc, in_maps, list(range(NC)))
    walls.append(time.time() - t0)

    out = np.zeros((N, D), np.float32)
    for c, core in enumerate(cores):
        r = res.results[c]["res"].reshape(nW, 128, D)
        base, nn = core["base"], core["nn"]
        for w in range(nW):
            if nn[w] > 0:
                out[base[w]:base[w] + nn[w]] = r[w, :nn[w]]
    if _return_walls:
        return out, walls
    return out


if __name__ == "__main__":
    d = np.load("/root/problem/refdata.npz")
    out = kernel(d["h_src"], d["h_dst"], d["W_fc"], d["w_attn"],
                 d["src"], d["dst"])
    exp = d["expected"]
    rel = np.linalg.norm(out - exp) / np.linalg.norm(exp)
    print(f"rel_l2 = {rel:.3e}  maxabs = {np.abs(out - exp).max():.3e}")
